# revision 1
# baseline (speedup 1.0000x reference)
"""CTAttention (dilated window attention) Trainium2 kernel.

Self-contained: hardcodes shapes from the problem spec.
  N=500000 tokens, C=256, H=8 heads (hd=32), window K=24, dilation D=4.
  Block = K*D = 96 tokens; attention is block-diagonal over dilated windows.

Sharding: blocks of 96 tokens across 8 cores (data parallel over windows).
Padded to 8*652 = 5216 blocks (real data needs 5209).
"""

import numpy as np

K = 24
D = 4
C = 256
H = 8
HD = 32
NTOK = 500000
BS = 8
BLOCK = K * D          # 96
NCORES = 8
NB = 652               # blocks per core
NBLKP = NCORES * NB    # 5216 padded blocks
TOK = NB * BLOCK       # 62592 tokens per core
NPAD = NBLKP * BLOCK   # 500736
SCALE = HD ** -0.5
SUP = 4                # groups (blocks) per supertile
NSUP = NB // SUP       # 163


def build_nc(nb):
    """Build the Bass program for `nb` blocks per core (nb % SUP == 0)."""
    import concourse.bacc as bacc
    import concourse.bass as bass
    import concourse.tile as tile
    from concourse import mybir

    f32 = mybir.dt.float32
    f32r = mybir.dt.float32r
    f16 = mybir.dt.float16
    AF = mybir.ActivationFunctionType
    OP = mybir.AluOpType

    nsup = nb // SUP
    tok = nb * BLOCK

    nc = bacc.Bacc("TRN2", target_bir_lowering=False, debug=False,
                   num_devices=NCORES)

    x = nc.declare_dram_parameter("x", [tok, C], f32, isOutput=False)
    mk_d = nc.declare_dram_parameter("mk", [nsup, BLOCK, SUP * BLOCK], f16,
                                     isOutput=False)
    wqkv_d = nc.declare_dram_parameter("wqkvT", [C, 3 * C], f32r,
                                       isOutput=False)
    bqk_d = nc.declare_dram_parameter("bqk", [128, 4], f32, isOutput=False)
    wp_d = nc.declare_dram_parameter("wpT", [C, C], f16, isOutput=False)
    beff_d = nc.declare_dram_parameter("beff", [C], f32, isOutput=False)
    id32_d = nc.declare_dram_parameter("id32", [BLOCK, BLOCK], f32,
                                       isOutput=False)
    id16_d = nc.declare_dram_parameter("id16", [BLOCK, BLOCK], f16,
                                       isOutput=False)
    y = nc.declare_dram_parameter("y", [tok, C], f32, isOutput=True)

    with tile.TileContext(nc) as tc:
        with (
            tc.tile_pool(name="const", bufs=1) as const,
            tc.tile_pool(name="xin", bufs=5) as xin_p,
            tc.tile_pool(name="xt", bufs=3) as xt_p,
            tc.tile_pool(name="qk", bufs=3) as qk_p,
            tc.tile_pool(name="mkp", bufs=3) as mk_p,
            tc.tile_pool(name="grp", bufs=10) as grp_p,
            tc.tile_pool(name="outp", bufs=10) as out_p,
            tc.tile_pool(name="ps", bufs=4, space="PSUM") as ps,
            tc.tile_pool(name="ps2", bufs=2, space="PSUM") as ps2,
        ):
            # ---- constants ----
            wq = const.tile([128, 2, 3 * C], f32r)
            nc.sync.dma_start(out=wq[:],
                              in_=wqkv_d.rearrange("(a p) f -> p a f", p=128))
            wp = const.tile([128, 2, C], f16)
            nc.sync.dma_start(out=wp[:],
                              in_=wp_d.rearrange("(a p) f -> p a f", p=128))
            bqk = const.tile([128, 4], f32)
            nc.sync.dma_start(out=bqk[:], in_=bqk_d[:, :])
            beff = const.tile([128, C], f32)
            nc.gpsimd.dma_start(out=beff[:],
                                in_=beff_d[None, :].to_broadcast((128, C)))
            id32 = const.tile([BLOCK, BLOCK], f32)
            nc.sync.dma_start(out=id32[:], in_=id32_d[:, :])
            id16 = const.tile([BLOCK, BLOCK], f16)
            nc.sync.dma_start(out=id16[:], in_=id16_d[:, :])

            def win_ap(t, b):
                # window-order view of block b: dims [(w:D), (k:K), (c:C)]
                return bass.AP(tensor=t, offset=b * BLOCK * C,
                               ap=[[C, D], [D * C, K], [1, C]])

            for it in range(nsup):
                # ---- load 4 blocks of X in window order: [96, 4, 256] ----
                xw = xin_p.tile([BLOCK, SUP, C], f32)
                for g in range(SUP):
                    b = it * SUP + g
                    # dest[(w,k), g, c] = x[b, k, w, c]
                    nc.sync.dma_start(out=xw[:, g, :], in_=win_ap(x, b))

                # ---- mask tile [96, 4, 96] ----
                mk = mk_p.tile([BLOCK, SUP, BLOCK], f16)
                nc.sync.dma_start(
                    out=mk[:],
                    in_=mk_d[it].rearrange("p (g j) -> p g j", g=SUP))

                # ---- X^T [128c, 2, 384] via PE transpose ----
                xt = xt_p.tile([128, 2, SUP * BLOCK], f32r)
                for g in range(SUP):
                    tp = ps.tile([128, 2, BLOCK], f32, tag="ps")
                    for cc in range(2):
                        nc.tensor.transpose(
                            tp[:, cc, :], xw[:, g, cc * 128:(cc + 1) * 128],
                            id32[:])
                    nc.vector.tensor_copy(
                        out=xt[:, :, g * BLOCK:(g + 1) * BLOCK], in_=tp[:])

                # ---- QK^T = W_qk @ X^T -> [128f, 4, 384] f16 (bias added) ----
                qk = qk_p.tile([128, 4, SUP * BLOCK], f16)
                for ft in range(4):
                    qps = ps.tile([128, SUP * BLOCK], f32, tag="ps")
                    for cc in range(2):
                        nc.tensor.matmul(
                            qps[:],
                            lhsT=wq[:, cc, ft * 128:(ft + 1) * 128],
                            rhs=xt[:, cc, :],
                            start=(cc == 0), stop=(cc == 1))
                    # bias-add on DVE keeps ACT exclusively on Exp (warm table)
                    nc.vector.tensor_scalar(
                        out=qk[:, ft, :], in0=qps[:],
                        scalar1=bqk[:, ft:ft + 1], scalar2=None,
                        op0=OP.add)

                for gp in range(SUP // 2):
                    # ---- S^T then P^T = exp(S^T) for a PAIR of groups ----
                    # 4 matmuls (2 heads x 2 groups) per psum tile are all on
                    # the same array strip (serialize) -> one bank is safe;
                    # one Exp per (strip, group-pair) halves ACT calls.
                    pt = grp_p.tile([BLOCK, 2, 4, 2, 128], f16)
                    nc.gpsimd.memset(pt[:, :, :, :, BLOCK:128], 0.0)
                    for h4 in range(4):
                        sp2 = ps.tile([BLOCK, 2, 2, BLOCK], f32, tag="ps")
                        rows = slice(32 * h4, 32 * h4 + 32)
                        for hh in range(2):
                            for gg in range(2):
                                gc = slice((2 * gp + gg) * BLOCK,
                                           (2 * gp + gg + 1) * BLOCK)
                                nc.tensor.matmul(
                                    sp2[:, hh, gg, :],
                                    lhsT=qk[rows, 2 + hh, gc],
                                    rhs=qk[rows, hh, gc],
                                    start=True, stop=True,
                                    tile_position=(32 * h4, 0))
                        nc.scalar.activation(
                            out=pt[:, :, h4, :, 0:BLOCK], in_=sp2[:],
                            func=AF.Exp, scale=1.0)
                    # ---- V for the pair: [96, 2, 256] (one bank) ----
                    vps = ps.tile([BLOCK, 2, C], f32, tag="ps")
                    for gg in range(2):
                        gc = slice((2 * gp + gg) * BLOCK,
                                   (2 * gp + gg + 1) * BLOCK)
                        for cc in range(2):
                            nc.tensor.matmul(
                                vps[:, gg, :],
                                lhsT=xt[:, cc, gc],
                                rhs=wq[:, cc, 2 * C:3 * C],
                                start=(cc == 0), stop=(cc == 1))
                    vv = grp_p.tile([BLOCK, 2, H, HD + 1], f16)
                    nc.vector.tensor_copy(
                        out=vv[:, :, :, 0:HD],
                        in_=vps.rearrange("p g (h d) -> p g h d", h=H))
                    nc.gpsimd.memset(vv[:, :, :, HD:HD + 1], 1.0)
                    for gg in range(2):
                        g = 2 * gp + gg
                        gcols = slice(g * BLOCK, (g + 1) * BLOCK)

                        # mask (broadcast over heads)
                        nc.vector.tensor_tensor(
                            out=pt[:, :, :, gg, 0:BLOCK],
                            in0=pt[:, :, :, gg, 0:BLOCK],
                            in1=mk[:, g, None, None, :].to_broadcast(
                                (BLOCK, 2, 4, BLOCK)),
                            op=OP.mult)

                        # ---- O' = P @ V' ; col 32 of each head = denom ----
                        # paired 2-bank psum: group gg at col gg*512
                        if gg == 0:
                            ops_t = ps2.tile([128, 2, 512], f32, tag="ps2")
                        for h in range(H):
                            nc.tensor.matmul(
                                ops_t[:, gg, h * (HD + 1):(h + 1) * (HD + 1)],
                                lhsT=pt[:, h // 4, h % 4, gg, :],
                                rhs=vv[:, gg, h, :],
                                start=True, stop=True)
                    opv = ops_t[:BLOCK, :, 0:H * (HD + 1)].rearrange(
                        "p g (h d) -> p g h d", h=H)
                    rc = grp_p.tile([BLOCK, 2, H], f32)
                    nc.vector.reciprocal(out=rc[:], in_=opv[:, :, :, HD])
                    og = grp_p.tile([BLOCK, 2, H, HD], f16)
                    nc.vector.tensor_tensor(
                        out=og[:],
                        in0=opv[:, :, :, 0:HD],
                        in1=rc[:, :, :, None].to_broadcast((BLOCK, 2, H, HD)),
                        op=OP.mult)
                    # ---- O^T via PE transpose, pair-batched (1 bank) ----
                    ot = grp_p.tile([128, 2, 2, BLOCK], f16)
                    otp = ps.tile([128, 2, 2, BLOCK], f16, tag="ps")
                    for gg in range(2):
                        for cc in range(2):
                            nc.tensor.transpose(
                                otp[:, gg, cc, :],
                                og[:, gg, 4 * cc:4 * cc + 4, :], id16[:])
                    nc.vector.tensor_copy(out=ot[:], in_=otp[:])

                    # ---- proj into paired 2-bank psum + one bias-add ----
                    fps = ps2.tile([BLOCK, 2, 512], f32, tag="ps2")
                    for gg in range(2):
                        for cc in range(2):
                            nc.tensor.matmul(
                                fps[:, gg, 0:C], lhsT=ot[:, gg, cc, :],
                                rhs=wp[:, cc, :],
                                start=(cc == 0), stop=(cc == 1))
                    yo = out_p.tile([BLOCK, 2, C], f32)
                    nc.vector.tensor_tensor(
                        out=yo[:], in0=fps[:, :, 0:C],
                        in1=beff[:BLOCK, None, :].to_broadcast((BLOCK, 2, C)),
                        op=OP.add)
                    for gg in range(2):
                        nc.sync.dma_start(
                            out=win_ap(y, it * SUP + 2 * gp + gg),
                            in_=yo[:, gg, :])

    nc.compile()
    return nc


def host_prep(data, qkv_w, qkv_b, proj_w, proj_b, batch_idx, ncores=NCORES,
              nb=NB):
    """Shard + preprocess inputs. Returns in_maps list for run_bass_kernel_spmd."""
    nblkp = ncores * nb
    npad = nblkp * BLOCK
    tok = nb * BLOCK
    nsup = nb // SUP

    n = data.shape[0]
    data_pad = np.zeros((npad, C), np.float32)
    data_pad[:n] = data
    batch_pad = np.full((npad,), BS, np.int32)
    batch_pad[:n] = batch_idx

    # categories in window order: block -> [k, w] -> win-order (w, k)
    cats = batch_pad.reshape(nblkp, K, D).transpose(0, 2, 1)  # [blk, w, k]
    cats = cats + 16 * np.arange(D, dtype=np.int32)[None, :, None]
    catw = cats.reshape(nblkp, BLOCK)
    mask01 = (catw[:, :, None] == catw[:, None, :]).astype(np.float16)
    # -> [core, nsup, 96, SUP*96] with layout [p, g, j]
    mk = (mask01.reshape(ncores, nsup, SUP, BLOCK, BLOCK)
          .transpose(0, 1, 3, 2, 4)
          .reshape(ncores, nsup, BLOCK, SUP * BLOCK).copy())

    wqkvT = np.ascontiguousarray(qkv_w.T).astype(np.float32).copy()
    wqkvT[:, :C] *= SCALE
    bqk_full = qkv_b[:2 * C].astype(np.float32).copy()
    bqk_full[:C] *= SCALE
    bqk = np.ascontiguousarray(bqk_full.reshape(4, 128).T)
    beff = (proj_b + qkv_b[2 * C:] @ proj_w.T).astype(np.float32)
    wpT = np.ascontiguousarray(proj_w.T).astype(np.float16)
    id32 = np.eye(BLOCK, dtype=np.float32)
    id16 = np.eye(BLOCK, dtype=np.float16)

    x_sh = data_pad.reshape(ncores, tok, C)
    in_maps = []
    for c in range(ncores):
        in_maps.append({
            "x": x_sh[c], "mk": mk[c], "wqkvT": wqkvT, "bqk": bqk,
            "wpT": wpT, "beff": beff, "id32": id32, "id16": id16,
        })
    return in_maps


_NC_CACHE = {}


def kernel(data, qkv_w, qkv_b, proj_w, proj_b, batch_idx):
    from concourse.bass_utils import run_bass_kernel_spmd

    data = np.asarray(data, np.float32)
    qkv_w = np.asarray(qkv_w, np.float32)
    qkv_b = np.asarray(qkv_b, np.float32)
    proj_w = np.asarray(proj_w, np.float32)
    proj_b = np.asarray(proj_b, np.float32)
    batch_idx = np.asarray(batch_idx, np.int32)

    if "nc" not in _NC_CACHE:
        _NC_CACHE["nc"] = build_nc(NB)
    nc = _NC_CACHE["nc"]

    in_maps = host_prep(data, qkv_w, qkv_b, proj_w, proj_b, batch_idx)
    res = run_bass_kernel_spmd(nc, in_maps, list(range(NCORES)))
    out = np.concatenate([res.results[c]["y"] for c in range(NCORES)], axis=0)
    return np.ascontiguousarray(out[:NTOK])

